# revision 19
# baseline (speedup 1.0000x reference)
"""Multi-head attention layer for Trainium2, 8 NeuronCores.

Problem (hardcoded): B=4, S=2048, D=1024, H=16 heads, DH=64.
  q,k,v = x@W* + b*;  scores = (q k^T)/sqrt(DH) - 10000*(1-mask_k);
  out = softmax(scores) @ v, heads concatenated.

Sharding: 8 cores = (batch b in 0..3) x (head-group g in 0..1).
Each core handles one batch element and 8 heads (512 of the 1024 output
channels), so outputs are disjoint and no collectives are needed.

Per-core kernel (all matmuls in fp32r = full-rate rounded fp32):
  1. x [S,D] is transposed on the PE into xT (d on partitions).
  2. QT/KT [dout, s] = W.T @ xT (bias via per-partition add on copy-out);
     V [s, dout] = xT.T @ W (bias via rank-1 ones matmul), stored per
     k-tile as V' = [V | 1] (extra ones column).
  3. Per head, per 512-query chunk: scoresT[k,q] accumulated per 128-k
     tile (two heads packed in one PE pass via row-group tiling);
     expT = Exp(0.125*scoresT + maskbias_k) on the scalar engine;
     h'T[dd,q] += V'[k,dd].T @ expT  (row 64 = sum of exp = softmax denom).
  4. h'T is transposed back on the PE; h = h'T[0:64]/h'T[64] streamed out.
"""
import numpy as np
from contextlib import ExitStack

import concourse.bass as bass
import concourse.bacc as bacc
import concourse.mybir as mybir
from concourse.tile import TileContext
from concourse.bass_utils import run_bass_kernel_spmd
from concourse.masks import make_identity

B, S, D, H = 4, 2048, 1024, 16
DH = 64
HPC = 8            # heads per core
DC = HPC * DH      # 512 output channels per core
KT_D = D // 128    # 8 contraction tiles over d_in
MT = DC // 128     # 4 tiles over local d_out
ST = S // 128      # 16 s-tiles
QCH = S // 512     # 4 query chunks
NCORES = 8

FP32 = mybir.dt.float32
FP32R = mybir.dt.float32r
BF16 = mybir.dt.bfloat16
AFT = mybir.ActivationFunctionType
import os
FP16 = mybir.dt.float16
_att = os.environ.get("ATTN_DT", "fp16")
ATTN_DT = {"bf16": BF16, "fp16": FP16, "fp32r": FP32R}[_att]
DUMMY_P2 = bool(os.environ.get("DUMMY_P2"))


def build_kernel():
    nc = bacc.Bacc("TRN2", target_bir_lowering=False, debug=False)
    x_d = nc.dram_tensor("x", (S, D), FP32, kind="ExternalInput")
    mask_d = nc.dram_tensor("mask", (S,), FP32, kind="ExternalInput")
    wq_d = nc.dram_tensor("wq", (D, DC), FP32, kind="ExternalInput")
    wk_d = nc.dram_tensor("wk", (D, DC), FP32, kind="ExternalInput")
    wv_d = nc.dram_tensor("wv", (D, DC), FP32, kind="ExternalInput")
    bq_d = nc.dram_tensor("bq", (DC,), FP32, kind="ExternalInput")
    bk_d = nc.dram_tensor("bk", (DC,), FP32, kind="ExternalInput")
    bv_d = nc.dram_tensor("bv", (DC,), FP32, kind="ExternalInput")
    out_d = nc.dram_tensor("out", (S, DC), FP32, kind="ExternalOutput")

    with TileContext(nc) as tc, ExitStack() as ctx:
        const = ctx.enter_context(tc.tile_pool(name="const", bufs=1))
        big = ctx.enter_context(tc.tile_pool(name="big", bufs=1))
        xt_pool = ctx.enter_context(tc.tile_pool(name="xtp", bufs=1))
        w_pool = ctx.enter_context(tc.tile_pool(name="wp", bufs=2))
        xin_pool = ctx.enter_context(tc.tile_pool(name="xinp", bufs=2))
        exp_pool = ctx.enter_context(tc.tile_pool(name="expp", bufs=6))
        ht_pool = ctx.enter_context(tc.tile_pool(name="htp", bufs=2))
        o_pool = ctx.enter_context(tc.tile_pool(name="op", bufs=2))
        ps_pool = ctx.enter_context(
            tc.tile_pool(name="psp", bufs=2, space=bass.MemorySpace.PSUM))
        psh_pool = ctx.enter_context(
            tc.tile_pool(name="pshp", bufs=2, space=bass.MemorySpace.PSUM))
        pst_pool = ctx.enter_context(
            tc.tile_pool(name="pstp", bufs=2, space=bass.MemorySpace.PSUM))

        ident = const.tile([128, 128], FP32)
        make_identity(nc, ident[:])
        ident_r = const.tile([128, 128], FP32R)
        nc.vector.tensor_copy(ident_r[:], ident[:])

        # mask -> additive bias per key position: -10000*(1-mask)
        mask_sb = const.tile([128, ST], FP32)
        nc.sync.dma_start(mask_sb[:], mask_d[:].rearrange("(t p) -> p t", p=128))
        kbias = const.tile([128, ST], FP32)
        nc.vector.tensor_scalar(kbias[:], mask_sb[:], -1.0, 10000.0,
                                mybir.AluOpType.add, mybir.AluOpType.mult)

        # projection biases
        bq_sb = const.tile([128, MT], FP32)
        bk_sb = const.tile([128, MT], FP32)
        nc.sync.dma_start(bq_sb[:], bq_d[:].rearrange("(m p) -> p m", p=128))
        nc.sync.dma_start(bk_sb[:], bk_d[:].rearrange("(m p) -> p m", p=128))
        bv_row = const.tile([1, DC], FP32R)
        nc.sync.dma_start(bv_row[:], bv_d[None, :].bitcast(FP32R))
        ones_f = const.tile([128, 128], FP32)
        nc.vector.memset(ones_f[:], 1.0)
        ones_r = const.tile([1, 128], FP32R)
        nc.vector.tensor_copy(ones_r[:], ones_f[0:1, :])

        # HAM warm-keeper: fp32r matmuls use the transpose-mode datapath and
        # do not register as PE activity, so the clock stays throttled at
        # 1.2GHz. A tiny zero-valued bf16 matmul accumulated into an live
        # PSUM group every ~1.5us keeps the PE at 2.4GHz.
        zb_bf = const.tile([128, 128], mybir.dt.bfloat16)
        nc.vector.memset(zb_bf[:], 0.0)
        db_rhs = const.tile([128, 64], mybir.dt.bfloat16)
        nc.vector.memset(db_rhs[:], 1.0)

        # persistent activations
        qt_sb = big.tile([128, MT, S], ATTN_DT)           # QT: [dout, s]
        kt_sb = big.tile([128, MT, S], ATTN_DT)           # KT: [dout, s]
        v_sb = big.tile([128, ST, HPC, DH + 1], ATTN_DT)  # V': [s_p, s_t, head, d|1]
        nc.vector.tensor_copy(
            v_sb[:, :, :, DH:DH + 1],
            ones_f[:].rearrange("p (a b c) -> p a b c", a=ST, b=HPC))

        # ---- phase 0: transpose x; phase 1: projections (single pass) ----
        xt_sb = xt_pool.tile([128, KT_D, S], FP32R, tag="xt")
        for st in range(ST):
            xin = xin_pool.tile([128, D], FP32R, tag="xin")
            nc.sync.dma_start(xin[:], x_d[st * 128:(st + 1) * 128, :].bitcast(FP32R))
            for dt_ in range(KT_D):
                tps = pst_pool.tile([128, 128], FP32R, tag="tp")
                nc.tensor.transpose(
                    tps[:], xin[:, dt_ * 128:(dt_ + 1) * 128], ident_r[:])
                nc.vector.tensor_copy(
                    xt_sb[:, dt_, st * 128:(st + 1) * 128], tps[:])

        # K before Q so attention (which needs all of K/V but only early Q
        # chunks) can start sooner.
        for w_d, b_sb, dst in ((wk_d, bk_sb, kt_sb), (wq_d, bq_sb, qt_sb)):
            w_sb = w_pool.tile([128, KT_D, DC], FP32R, tag="w")
            nc.sync.dma_start(
                w_sb[:], w_d[:].rearrange("(k p) n -> p k n", p=128).bitcast(FP32R))
            for mt in range(MT):
                for qch in range(QCH):
                    ps = ps_pool.tile([128, 512], FP32, tag="ps")
                    for kt in range(KT_D):
                        nc.tensor.matmul(
                            ps[:],
                            w_sb[:, kt, mt * 128:(mt + 1) * 128],
                            xt_sb[:, kt, qch * 512:(qch + 1) * 512],
                            start=(kt == 0), stop=(kt == KT_D - 1))
                        if kt == 0:
                            nc.tensor.matmul(ps[:, 0:64], zb_bf[:],
                                             db_rhs[:], start=False, stop=False)
                    nc.vector.tensor_scalar_add(
                        dst[:, mt, qch * 512:(qch + 1) * 512],
                        ps[:], b_sb[:, mt:mt + 1])

        # V projection (natural orientation) + bias via rank-1 matmul
        wv_sb = w_pool.tile([128, KT_D, DC], FP32R, tag="w")
        nc.sync.dma_start(
            wv_sb[:], wv_d[:].rearrange("(k p) n -> p k n", p=128).bitcast(FP32R))
        for st in range(ST):
            ps = ps_pool.tile([128, 512], FP32, tag="ps")
            for kt in range(KT_D):
                nc.tensor.matmul(
                    ps[:],
                    xt_sb[:, kt, st * 128:(st + 1) * 128],
                    wv_sb[:, kt, :],
                    start=(kt == 0), stop=False)
                if kt == 0:
                    nc.tensor.matmul(ps[:, 0:64], zb_bf[:],
                                     db_rhs[:], start=False, stop=False)
            nc.tensor.matmul(ps[:], ones_r[:], bv_row[:], start=False, stop=True)
            nc.vector.tensor_copy(
                v_sb[:, st, :, 0:DH],
                ps[:].rearrange("p (h d) -> p h d", d=DH))

        # ---- phase 2: attention, two heads (one pair) per PE pass ----
        for pair in range(HPC // 2):
            for qc in range(QCH):
                q0 = qc * 512
                hA = psh_pool.tile([DH + 1, 512], FP32, tag="h")
                hB = psh_pool.tile([DH + 1, 512], FP32, tag="h")
                # software pipeline (depth 2): pv(kt-2) is emitted before
                # scores(kt) so the PE never waits on the scalar engine's
                # exp, and the two K=64 score matmuls stay adjacent
                # (disjoint row groups overlap in the array).
                from collections import deque
                pend = deque()
                def flush_pv(last=False):
                    pkt, pe = pend.popleft()
                    nc.tensor.matmul(hA[:], v_sb[:, pkt, 2 * pair, :],
                                     pe[:, 0:512],
                                     start=(pkt == 0), stop=last and not pend)
                    nc.tensor.matmul(hB[:], v_sb[:, pkt, 2 * pair + 1, :],
                                     pe[:, 512:1024],
                                     start=(pkt == 0), stop=last and not pend)
                    if DUMMY_P2 and pkt >= 1 and pkt < ST - 1:
                        nc.tensor.matmul(hA[:, 0:64], zb_bf[:, 0:DH + 1],
                                         db_rhs[:], start=False, stop=False)
                for kt in range(ST):
                    k0 = kt * 128
                    if len(pend) >= 2:
                        flush_pv()
                    scAB = ps_pool.tile([128, 1024], FP32, tag="ps")
                    nc.tensor.matmul(scAB[:, 0:512], kt_sb[0:64, pair, k0:k0 + 128],
                                     qt_sb[0:64, pair, q0:q0 + 512],
                                     start=True, stop=True)
                    nc.tensor.matmul(scAB[:, 512:1024], kt_sb[64:128, pair, k0:k0 + 128],
                                     qt_sb[64:128, pair, q0:q0 + 512],
                                     start=True, stop=True)
                    eAB = exp_pool.tile([128, 1024], ATTN_DT, tag="exp")
                    nc.scalar.activation(eAB[:], scAB[:], AFT.Exp,
                                         bias=kbias[:, kt:kt + 1], scale=0.125)
                    pend.append((kt, eAB))
                while pend:
                    flush_pv(last=True)

                for hl, h_ps in ((2 * pair, hA), (2 * pair + 1, hB)):
                    ht_sb = ht_pool.tile([DH + 1, 512], FP32, tag="ht")
                    nc.vector.tensor_copy(ht_sb[:], h_ps[:])
                    for qt in range(4):
                        tps = pst_pool.tile([128, DH + 1], FP32, tag="tp")
                        nc.tensor.transpose(
                            tps[:], ht_sb[:, qt * 128:(qt + 1) * 128],
                            ident[0:DH + 1, 0:DH + 1])
                        rec = o_pool.tile([128, 1], FP32, tag="rec")
                        nc.vector.reciprocal(rec[:], tps[:, DH:DH + 1])
                        o_sb = o_pool.tile([128, DH], FP32, tag="o")
                        nc.vector.tensor_scalar_mul(o_sb[:], tps[:, 0:DH], rec[:])
                        row = q0 + qt * 128
                        nc.sync.dma_start(
                            out_d[row:row + 128, hl * DH:(hl + 1) * DH], o_sb[:])

    nc.compile()
    return nc


_NC_CACHE = None


def _get_nc():
    global _NC_CACHE
    if _NC_CACHE is None:
        _NC_CACHE = build_kernel()
    return _NC_CACHE


def make_in_maps(x, mask, Wq, bq, Wk, bk, Wv, bv):
    asc = np.ascontiguousarray
    in_maps = []
    for c in range(NCORES):
        b, g = divmod(c, 2)
        cs = slice(g * DC, (g + 1) * DC)
        in_maps.append({
            "x": asc(x[b], dtype=np.float32),
            "mask": asc(mask[b], dtype=np.float32),
            "wq": asc(Wq[:, cs], dtype=np.float32),
            "wk": asc(Wk[:, cs], dtype=np.float32),
            "wv": asc(Wv[:, cs], dtype=np.float32),
            "bq": asc(bq[cs], dtype=np.float32),
            "bk": asc(bk[cs], dtype=np.float32),
            "bv": asc(bv[cs], dtype=np.float32),
        })
    return in_maps


def kernel(x, mask, Wq, bq, Wk, bk, Wv, bv):
    nc = _get_nc()
    in_maps = make_in_maps(x, mask, Wq, bq, Wk, bk, Wv, bv)
    res = run_bass_kernel_spmd(nc, in_maps, core_ids=list(range(NCORES)))
    out = np.empty((B, S, D), dtype=np.float32)
    for c in range(NCORES):
        b, g = divmod(c, 2)
        out[b, :, g * DC:(g + 1) * DC] = res.results[c]["out"]
    return out


# revision 20
# speedup vs baseline: 1.0400x; 1.0400x over previous
"""Multi-head attention layer for Trainium2, 8 NeuronCores.

Problem (hardcoded): B=4, S=2048, D=1024, H=16 heads, DH=64.
  q,k,v = x@W* + b*;  scores = (q k^T)/sqrt(DH) - 10000*(1-mask_k);
  out = softmax(scores) @ v, heads concatenated.

Sharding: 8 cores = (batch b in 0..3) x (head-group g in 0..1).
Each core handles one batch element and 8 heads (512 of the 1024 output
channels), so outputs are disjoint and no collectives are needed.

Per-core kernel (all matmuls in fp32r = full-rate rounded fp32):
  1. x [S,D] is transposed on the PE into xT (d on partitions).
  2. QT/KT [dout, s] = W.T @ xT (bias via per-partition add on copy-out);
     V [s, dout] = xT.T @ W (bias via rank-1 ones matmul), stored per
     k-tile as V' = [V | 1] (extra ones column).
  3. Per head, per 512-query chunk: scoresT[k,q] accumulated per 128-k
     tile (two heads packed in one PE pass via row-group tiling);
     expT = Exp(0.125*scoresT + maskbias_k) on the scalar engine;
     h'T[dd,q] += V'[k,dd].T @ expT  (row 64 = sum of exp = softmax denom).
  4. h'T is transposed back on the PE; h = h'T[0:64]/h'T[64] streamed out.
"""
import numpy as np
from contextlib import ExitStack

import concourse.bass as bass
import concourse.bacc as bacc
import concourse.mybir as mybir
from concourse.tile import TileContext
from concourse.bass_utils import run_bass_kernel_spmd
from concourse.masks import make_identity

B, S, D, H = 4, 2048, 1024, 16
DH = 64
HPC = 8            # heads per core
DC = HPC * DH      # 512 output channels per core
KT_D = D // 128    # 8 contraction tiles over d_in
MT = DC // 128     # 4 tiles over local d_out
ST = S // 128      # 16 s-tiles
QCH = S // 512     # 4 query chunks
NCORES = 8

FP32 = mybir.dt.float32
FP32R = mybir.dt.float32r
BF16 = mybir.dt.bfloat16
AFT = mybir.ActivationFunctionType
import os
FP16 = mybir.dt.float16
_att = os.environ.get("ATTN_DT", "fp16")
ATTN_DT = {"bf16": BF16, "fp16": FP16, "fp32r": FP32R}[_att]
DUMMY_P2 = bool(os.environ.get("DUMMY_P2"))


def build_kernel():
    nc = bacc.Bacc("TRN2", target_bir_lowering=False, debug=False)
    x_d = nc.dram_tensor("x", (S, D), FP32, kind="ExternalInput")
    mask_d = nc.dram_tensor("mask", (S,), FP32, kind="ExternalInput")
    wq_d = nc.dram_tensor("wq", (D, DC), FP32, kind="ExternalInput")
    wk_d = nc.dram_tensor("wk", (D, DC), FP32, kind="ExternalInput")
    wv_d = nc.dram_tensor("wv", (D, DC), FP32, kind="ExternalInput")
    bq_d = nc.dram_tensor("bq", (DC,), FP32, kind="ExternalInput")
    bk_d = nc.dram_tensor("bk", (DC,), FP32, kind="ExternalInput")
    bv_d = nc.dram_tensor("bv", (DC,), FP32, kind="ExternalInput")
    out_d = nc.dram_tensor("out", (S, DC), FP32, kind="ExternalOutput")

    with TileContext(nc) as tc, ExitStack() as ctx:
        const = ctx.enter_context(tc.tile_pool(name="const", bufs=1))
        big = ctx.enter_context(tc.tile_pool(name="big", bufs=1))
        xt_pool = ctx.enter_context(tc.tile_pool(name="xtp", bufs=1))
        w_pool = ctx.enter_context(tc.tile_pool(name="wp", bufs=2))
        xin_pool = ctx.enter_context(tc.tile_pool(name="xinp", bufs=2))
        exp_pool = ctx.enter_context(tc.tile_pool(name="expp", bufs=6))
        ht_pool = ctx.enter_context(tc.tile_pool(name="htp", bufs=2))
        o_pool = ctx.enter_context(tc.tile_pool(name="op", bufs=2))
        ps_pool = ctx.enter_context(
            tc.tile_pool(name="psp", bufs=2, space=bass.MemorySpace.PSUM))
        psh_pool = ctx.enter_context(
            tc.tile_pool(name="pshp", bufs=2, space=bass.MemorySpace.PSUM))
        pst_pool = ctx.enter_context(
            tc.tile_pool(name="pstp", bufs=2, space=bass.MemorySpace.PSUM))

        ident = const.tile([128, 128], FP32)
        make_identity(nc, ident[:])
        ident_r = const.tile([128, 128], FP32R)
        nc.vector.tensor_copy(ident_r[:], ident[:])

        # mask -> additive bias per key position: -10000*(1-mask)
        mask_sb = const.tile([128, ST], FP32)
        nc.sync.dma_start(mask_sb[:], mask_d[:].rearrange("(t p) -> p t", p=128))
        kbias = const.tile([128, ST], FP32)
        nc.vector.tensor_scalar(kbias[:], mask_sb[:], -1.0, 10000.0,
                                mybir.AluOpType.add, mybir.AluOpType.mult)

        # projection biases
        bq_sb = const.tile([128, MT], FP32)
        bk_sb = const.tile([128, MT], FP32)
        nc.sync.dma_start(bq_sb[:], bq_d[:].rearrange("(m p) -> p m", p=128))
        nc.sync.dma_start(bk_sb[:], bk_d[:].rearrange("(m p) -> p m", p=128))
        bv_row = const.tile([1, DC], FP32R)
        nc.sync.dma_start(bv_row[:], bv_d[None, :].bitcast(FP32R))
        ones_f = const.tile([128, 128], FP32)
        nc.vector.memset(ones_f[:], 1.0)
        ones_r = const.tile([1, 128], FP32R)
        nc.vector.tensor_copy(ones_r[:], ones_f[0:1, :])

        # HAM warm-keeper: fp32r matmuls use the transpose-mode datapath and
        # do not register as PE activity, so the clock stays throttled at
        # 1.2GHz. A tiny zero-valued bf16 matmul accumulated into an live
        # PSUM group every ~1.5us keeps the PE at 2.4GHz.
        zb_bf = const.tile([128, 128], mybir.dt.bfloat16)
        nc.vector.memset(zb_bf[:], 0.0)
        db_rhs = const.tile([128, 64], mybir.dt.bfloat16)
        nc.vector.memset(db_rhs[:], 1.0)

        # persistent activations
        qt_sb = big.tile([128, MT, S], ATTN_DT)           # QT: [dout, s]
        kt_sb = big.tile([128, MT, S], ATTN_DT)           # KT: [dout, s]
        v_sb = big.tile([128, ST, HPC, DH + 1], ATTN_DT)  # V': [s_p, s_t, head, d|1]
        nc.vector.tensor_copy(
            v_sb[:, :, :, DH:DH + 1],
            ones_f[:].rearrange("p (a b c) -> p a b c", a=ST, b=HPC))

        # ---- phase 0: transpose x; phase 1: projections (single pass) ----
        xt_sb = xt_pool.tile([128, KT_D, S], FP32R, tag="xt")
        for st in range(ST):
            xin = xin_pool.tile([128, D], FP32R, tag="xin")
            nc.sync.dma_start(xin[:], x_d[st * 128:(st + 1) * 128, :].bitcast(FP32R))
            for dt_ in range(KT_D):
                tps = pst_pool.tile([128, 128], FP32R, tag="tp")
                nc.tensor.transpose(
                    tps[:], xin[:, dt_ * 128:(dt_ + 1) * 128], ident_r[:])
                nc.vector.tensor_copy(
                    xt_sb[:, dt_, st * 128:(st + 1) * 128], tps[:])

        # V projection first (natural orientation) + bias via rank-1 matmul
        wv_sb = w_pool.tile([128, KT_D, DC], FP32R, tag="w")
        nc.sync.dma_start(
            wv_sb[:], wv_d[:].rearrange("(k p) n -> p k n", p=128).bitcast(FP32R))
        wk_sb = w_pool.tile([128, KT_D, DC], FP32R, tag="w")
        nc.sync.dma_start(
            wk_sb[:], wk_d[:].rearrange("(k p) n -> p k n", p=128).bitcast(FP32R))
        for st in range(ST):
            ps = ps_pool.tile([128, 512], FP32, tag="ps")
            for kt in range(KT_D):
                nc.tensor.matmul(
                    ps[:],
                    xt_sb[:, kt, st * 128:(st + 1) * 128],
                    wv_sb[:, kt, :],
                    start=(kt == 0), stop=False)
                if kt == 0:
                    nc.tensor.matmul(ps[:, 0:64], zb_bf[:],
                                     db_rhs[:], start=False, stop=False)
            nc.tensor.matmul(ps[:], ones_r[:], bv_row[:], start=False, stop=True)
            nc.vector.tensor_copy(
                v_sb[:, st, :, 0:DH],
                ps[:].rearrange("p (h d) -> p h d", d=DH))
        wq_sb = w_pool.tile([128, KT_D, DC], FP32R, tag="w")
        nc.sync.dma_start(
            wq_sb[:], wq_d[:].rearrange("(k p) n -> p k n", p=128).bitcast(FP32R))

        def project_kq(mt):
            for w_sb, b_sb, dst in ((wk_sb, bk_sb, kt_sb), (wq_sb, bq_sb, qt_sb)):
                for qch in range(QCH):
                    ps = ps_pool.tile([128, 512], FP32, tag="ps")
                    for kt in range(KT_D):
                        nc.tensor.matmul(
                            ps[:],
                            w_sb[:, kt, mt * 128:(mt + 1) * 128],
                            xt_sb[:, kt, qch * 512:(qch + 1) * 512],
                            start=(kt == 0), stop=(kt == KT_D - 1))
                        if kt == 0:
                            nc.tensor.matmul(ps[:, 0:64], zb_bf[:],
                                             db_rhs[:], start=False, stop=False)
                    nc.vector.tensor_scalar_add(
                        dst[:, mt, qch * 512:(qch + 1) * 512],
                        ps[:], b_sb[:, mt:mt + 1])

        # ---- phase 2: attention; K/Q are projected per head-pair just
        # before that pair's attention so pair p+1's projections (PE) run
        # while pair p's exps keep the scalar engine busy. ----
        for pair in range(HPC // 2):
            project_kq(pair)
            for qc in range(QCH):
                q0 = qc * 512
                hA = psh_pool.tile([DH + 1, 512], FP32, tag="h")
                hB = psh_pool.tile([DH + 1, 512], FP32, tag="h")
                # software pipeline (depth 2): pv(kt-2) is emitted before
                # scores(kt) so the PE never waits on the scalar engine's
                # exp, and the two K=64 score matmuls stay adjacent
                # (disjoint row groups overlap in the array).
                from collections import deque
                pend = deque()
                def flush_pv(last=False):
                    pkt, pe = pend.popleft()
                    nc.tensor.matmul(hA[:], v_sb[:, pkt, 2 * pair, :],
                                     pe[:, 0:512],
                                     start=(pkt == 0), stop=last and not pend)
                    nc.tensor.matmul(hB[:], v_sb[:, pkt, 2 * pair + 1, :],
                                     pe[:, 512:1024],
                                     start=(pkt == 0), stop=last and not pend)
                    if DUMMY_P2 and pkt >= 1 and pkt < ST - 1:
                        nc.tensor.matmul(hA[:, 0:64], zb_bf[:, 0:DH + 1],
                                         db_rhs[:], start=False, stop=False)
                for kt in range(ST):
                    k0 = kt * 128
                    if len(pend) >= 2:
                        flush_pv()
                    scAB = ps_pool.tile([128, 1024], FP32, tag="ps")
                    nc.tensor.matmul(scAB[:, 0:512], kt_sb[0:64, pair, k0:k0 + 128],
                                     qt_sb[0:64, pair, q0:q0 + 512],
                                     start=True, stop=True)
                    nc.tensor.matmul(scAB[:, 512:1024], kt_sb[64:128, pair, k0:k0 + 128],
                                     qt_sb[64:128, pair, q0:q0 + 512],
                                     start=True, stop=True)
                    eAB = exp_pool.tile([128, 1024], ATTN_DT, tag="exp")
                    nc.scalar.activation(eAB[:], scAB[:], AFT.Exp,
                                         bias=kbias[:, kt:kt + 1], scale=0.125)
                    pend.append((kt, eAB))
                while pend:
                    flush_pv(last=True)

                for hl, h_ps in ((2 * pair, hA), (2 * pair + 1, hB)):
                    ht_sb = ht_pool.tile([DH + 1, 512], FP32, tag="ht")
                    nc.vector.tensor_copy(ht_sb[:], h_ps[:])
                    for qt in range(4):
                        tps = pst_pool.tile([128, DH + 1], FP32, tag="tp")
                        nc.tensor.transpose(
                            tps[:], ht_sb[:, qt * 128:(qt + 1) * 128],
                            ident[0:DH + 1, 0:DH + 1])
                        rec = o_pool.tile([128, 1], FP32, tag="rec")
                        nc.vector.reciprocal(rec[:], tps[:, DH:DH + 1])
                        o_sb = o_pool.tile([128, DH], FP32, tag="o")
                        nc.vector.tensor_scalar_mul(o_sb[:], tps[:, 0:DH], rec[:])
                        row = q0 + qt * 128
                        nc.sync.dma_start(
                            out_d[row:row + 128, hl * DH:(hl + 1) * DH], o_sb[:])

    nc.compile()
    return nc


_NC_CACHE = None


def _get_nc():
    global _NC_CACHE
    if _NC_CACHE is None:
        _NC_CACHE = build_kernel()
    return _NC_CACHE


def make_in_maps(x, mask, Wq, bq, Wk, bk, Wv, bv):
    asc = np.ascontiguousarray
    in_maps = []
    for c in range(NCORES):
        b, g = divmod(c, 2)
        cs = slice(g * DC, (g + 1) * DC)
        in_maps.append({
            "x": asc(x[b], dtype=np.float32),
            "mask": asc(mask[b], dtype=np.float32),
            "wq": asc(Wq[:, cs], dtype=np.float32),
            "wk": asc(Wk[:, cs], dtype=np.float32),
            "wv": asc(Wv[:, cs], dtype=np.float32),
            "bq": asc(bq[cs], dtype=np.float32),
            "bk": asc(bk[cs], dtype=np.float32),
            "bv": asc(bv[cs], dtype=np.float32),
        })
    return in_maps


def kernel(x, mask, Wq, bq, Wk, bk, Wv, bv):
    nc = _get_nc()
    in_maps = make_in_maps(x, mask, Wq, bq, Wk, bk, Wv, bv)
    res = run_bass_kernel_spmd(nc, in_maps, core_ids=list(range(NCORES)))
    out = np.empty((B, S, D), dtype=np.float32)
    for c in range(NCORES):
        b, g = divmod(c, 2)
        out[b, :, g * DC:(g + 1) * DC] = res.results[c]["out"]
    return out
